# revision 15
# baseline (speedup 1.0000x reference)
"""DeepAndWide Trainium2 kernel (8 NeuronCores, SPMD via Bass/Tile).

Math:
    g = concat(hs, ht, 1) @ W_L.T + b_L            # [B, 2D] deep path
    v = outer(hs_b, ht_b) flattened -> [B, D*D]    # wide-path features
    u = v @ W_L2.T + b_L2                          # [B, 2D]
    out = concat(g, u, 1)                          # [B, 4D]

Sharding: W_L2 is column-sharded over the D*D contraction dim across 8
cores (each core owns 48 of the 384 outer-product rows i); every core
computes a partial u over the full batch; the host sums the partials.
g is data-parallel over batch (128 rows per core).

On-chip per core: vT tiles ([k,b] layout) are built on the VectorEngine as
htT[j-block] * broadcast(hs[:, i]); the broadcast across partitions is
pre-materialized on the host (HSB). Matmuls run in float32r (full PE rate,
~2e-4 relative error) with W2 chunks stationary ([128k,128o]) and vT
moving (N=512). Output is accumulated transposed (uT) in PSUM, flushed
per 3-i group into an SBUF fp32 accumulator.
"""

import numpy as np

import concourse.bass as bass
import concourse.mybir as mybir
from concourse import tile
from concourse.bass_utils import run_bass_kernel_spmd

B = 1024
D = 384
NCORES = 8
IC = D // NCORES          # 48 outer-product rows (i) per core
KC = IC * D               # 18432 contraction columns per core
TWO_D = 2 * D             # 768

KG_I = 3                  # i's per flush group
NKG = IC // KG_I          # 16 flush groups
TKG = KG_I * 3            # 9 k-tiles (of 128) per flush group

CDT = mybir.dt.float32r   # matmul compute dtype
F32 = mybir.dt.float32

# exec_time_ns of the last hardware run (set when BASS_TRACE=1)
LAST_EXEC_TIME_NS = None
LAST_RESULTS = None


def _split_excess_waits(nc):
    """walrus rejects >1 sync-wait on several instruction structs (fp32/f32r
    Matmult, Drain, ...). Hoist all but the last wait of any multi-wait
    instruction onto single-wait EventSemaphore instructions inserted just
    before it on the same engine."""
    n = [0]

    def fresh():
        n[0] += 1
        return f"WSPLIT-{n[0]}"

    for f in nc.m.functions:
        for blk in f.blocks:
            out = []
            changed = False
            for ins in blk.instructions:
                si = ins.sync_info
                if si is not None and len(si.on_wait) > 1:
                    waits = list(si.on_wait)
                    for w in waits[:-1]:
                        ev = mybir.InstEventSemaphore(
                            name=fresh(),
                            engine=ins.engine,
                            ins=[],
                            outs=[],
                            sync_info=mybir.SyncInfo(on_wait=[w], on_update=[]),
                        )
                        out.append(ev)
                    ins.sync_info = mybir.SyncInfo(
                        on_wait=[waits[-1]], on_update=list(si.on_update)
                    )
                    changed = True
                out.append(ins)
            if changed:
                blk.instructions = out


def _strip_unused_mm_incs(nc):
    """Every matmul carries a +1 update on the PE semaphore; the EVT_SEM
    register write costs the PE queue ~26ns each. Keep only the increments
    whose cumulative tick some wait actually references (plus the final
    one), and renumber all waits on that semaphore accordingly."""
    from collections import defaultdict

    for f in nc.m.functions:
        upd_insts = defaultdict(list)
        wait_refs = defaultdict(list)
        for blk in f.blocks:
            for ins in blk.instructions:
                si = ins.sync_info
                if not si:
                    continue
                for u in si.on_update:
                    upd_insts[u.id].append((ins, u))
                for w in si.on_wait:
                    wait_refs[w.id].append(w)

        for sem_id, upds in upd_insts.items():
            if not all(
                type(i).__name__ == "InstMatmult"
                and u.update_mode == "sem-inc"
                and u.update_value == 1
                for i, u in upds
            ):
                continue
            ws = wait_refs.get(sem_id, [])
            if any(
                w.wait_mode != "sem-ge-imm" or w.wait_reg is not None for w in ws
            ):
                continue
            used = {w.wait_value for w in ws}
            n = len(upds)
            keep = []
            kept_prefix = []
            kept = 0
            for tick in range(1, n + 1):
                k = tick in used or tick == n
                keep.append(k)
                kept += 1 if k else 0
                kept_prefix.append(kept)
            for (ins, u), k in zip(upds, keep):
                if not k:
                    si = ins.sync_info
                    ins.sync_info = mybir.SyncInfo(
                        on_wait=list(si.on_wait),
                        on_update=[x for x in si.on_update if x.id != sem_id],
                    )
            for w in ws:
                v = w.wait_value
                if v >= 1:
                    w.wait_value = kept_prefix[min(v, n) - 1]


def _gen():
    nc = bass.Bass()

    w2t = nc.dram_tensor("w2t", [KC, TWO_D], CDT, kind="ExternalInput")
    hsb = nc.dram_tensor("hsb", [IC, 128, B], CDT, kind="ExternalInput")
    htt = nc.dram_tensor("htt", [D, B], CDT, kind="ExternalInput")
    xt = nc.dram_tensor("xt", [TWO_D, 128], CDT, kind="ExternalInput")
    wlt = nc.dram_tensor("wlt", [TWO_D, TWO_D], CDT, kind="ExternalInput")
    u_out = nc.dram_tensor("u_out", [TWO_D, B], F32, kind="ExternalOutput")  # uT
    g_out = nc.dram_tensor("g_out", [128, TWO_D], F32, kind="ExternalOutput")

    w2t_r = w2t.rearrange("(g t p) o -> g t p o", t=TKG, p=128)   # [NKG,TKG,128,2D]
    hsb_r = hsb.rearrange("(g i) p b -> g i p b", i=KG_I)         # [NKG,KG_I,128,B]
    htt_r = htt.rearrange("(j p) b -> j p b", p=128)              # [3,128,B]
    u_out_r = u_out.rearrange("(c p) b -> c p b", p=128)          # [6,128,B]

    with tile.TileContext(nc) as tc:
        with (
            tc.tile_pool(name="const", bufs=1) as constp,
            tc.tile_pool(name="w2p", bufs=3) as w2p,
            tc.tile_pool(name="hsbp", bufs=2) as hsbp,
            tc.tile_pool(name="vtp", bufs=2) as vtp,
            tc.tile_pool(name="psump", bufs=3, space="PSUM") as psump,
            tc.tile_pool(name="gpool", bufs=1) as gp,
            tc.tile_pool(name="gpsum", bufs=1, space="PSUM") as gps,
        ):
            wlt_t = gp.tile([128, 6, TWO_D], CDT, tag="wlt")
            xt_t = gp.tile([128, 6, 128], CDT, tag="xtt")
            g_sb = gp.tile([128, TWO_D], F32, tag="gsb")

            htt_t = constp.tile([128, 3, B], CDT, tag="htt")
            # uT accumulator: uacc[p, oc, b] = u[b, oc*128 + p]
            uacc = constp.tile([128, 6, B], F32, tag="uacc")

            # critical-path loads first; issue spread over idle engine
            # queues (DMA issue serializes at ~0.6us per op per engine)
            for jb in range(3):
                nc.scalar.dma_start(htt_t[:, jb, :], htt_r[jb])

            # i-group schedule: ramp up so early compute tracks DMA arrival
            groups = [1, 1, 1, 2, 2, 2] + [KG_I] * (NKG - 3)
            assert sum(groups) == IC
            i0 = 0
            for gi, gsz in enumerate(groups):
                nt = gsz * 3                    # k-tiles in this group
                w2_t = w2p.tile([128, nt, TWO_D], CDT, tag="w2")
                hsb_t = hsbp.tile([128, gsz, B], CDT, tag="hsb")
                for il in range(gsz):
                    nc.gpsimd.dma_start(
                        hsb_t[:, il, :],
                        hsb.rearrange("i p b -> i p b")[i0 + il],
                    )
                w2_flat = w2t.rearrange("(kt p) o -> kt p o", p=128)
                if gi < 7:
                    # fine granularity so the first matmuls start early
                    for t in range(nt):
                        nc.sync.dma_start(w2_t[:, t, :], w2_flat[i0 * 3 + t])
                else:
                    nc.sync.dma_start(
                        w2_t[:],
                        w2t.rearrange("(kt p) o -> kt p o", p=128)[
                            i0 * 3 : i0 * 3 + nt
                        ].rearrange("t p o -> p t o"),
                    )
                if gi == 8:
                    # g-path inputs; g compute happens near the end
                    nc.scalar.dma_start(
                        wlt_t[:], wlt.rearrange("(t p) o -> p t o", p=128)
                    )
                    nc.scalar.dma_start(
                        xt_t[:], xt.rearrange("(t p) b -> p t b", p=128)
                    )

                for bh in range(2):
                    bsl = slice(bh * 512, (bh + 1) * 512)
                    vt = vtp.tile([128, nt, 512], CDT, tag="vt")
                    for t in range(nt):
                        il, jb = divmod(t, 3)
                        nc.vector.tensor_mul(
                            vt[:, t, :], htt_t[:, jb, bsl], hsb_t[:, il, bsl]
                        )
                    # lhsT = W2 chunk [k,128o] (stationary), rhs = vT [k,512b]
                    # out = uT chunk [128o, 512b]; PSUM tile holds 2 o-chunks.
                    # k-tiles consumed in chunks of 3 so weight DMA arrival
                    # is just-in-time instead of whole-group bursts.
                    ps0 = psump.tile([128, 2, 512], F32, tag="ps")
                    ps1 = psump.tile([128, 2, 512], F32, tag="ps")
                    ps2 = psump.tile([128, 2, 512], F32, tag="ps")
                    pss = [ps0, ps1, ps2]
                    for tp in range(gsz):
                        for op_ in range(3):
                            for half in range(2):
                                oc = op_ * 2 + half
                                for tt in range(3):
                                    t = tp * 3 + tt
                                    nc.tensor.matmul(
                                        pss[op_][:, half, :],
                                        w2_t[:, t, oc * 128 : (oc + 1) * 128],
                                        vt[:, t, :],
                                        start=(t == 0),
                                        stop=(t == nt - 1),
                                    )
                    for op_ in range(3):
                        if gi == 0:
                            nc.vector.tensor_copy(
                                uacc[:, op_ * 2 : op_ * 2 + 2, bsl], pss[op_][:]
                            )
                        else:
                            nc.vector.tensor_add(
                                uacc[:, op_ * 2 : op_ * 2 + 2, bsl],
                                uacc[:, op_ * 2 : op_ * 2 + 2, bsl],
                                pss[op_][:],
                            )

                if gi == len(groups) - 2:
                    # deep path compute slotted before the last group so it
                    # stays off the kernel tail
                    g_ps = gps.tile([128, TWO_D], F32, tag="gps")
                    for t in range(6):
                        nc.tensor.matmul(
                            g_ps[:, 0:512],
                            xt_t[:, t, :],
                            wlt_t[:, t, 0:512],
                            start=(t == 0),
                            stop=(t == 5),
                        )
                        nc.tensor.matmul(
                            g_ps[:, 512:768],
                            xt_t[:, t, :],
                            wlt_t[:, t, 512:768],
                            start=(t == 0),
                            stop=(t == 5),
                        )
                    nc.vector.tensor_copy(g_sb[:], g_ps[:])
                    nc.sync.dma_start(g_out[:], g_sb[:])
                i0 += gsz

            for oc in range(6):
                nc.sync.dma_start(u_out_r[oc], uacc[:, oc, :])

    _split_excess_waits(nc)
    _strip_unused_mm_incs(nc)
    return nc


_NC_CACHE = None


def kernel(hspatial, htext, W_L, b_L, W_L2, b_L2):
    global LAST_EXEC_TIME_NS, LAST_RESULTS, _NC_CACHE

    hs = np.asarray(hspatial, dtype=np.float32)
    ht = np.asarray(htext, dtype=np.float32)
    W_L = np.asarray(W_L, dtype=np.float32)
    b_L = np.asarray(b_L, dtype=np.float32)
    W_L2 = np.asarray(W_L2, dtype=np.float32)
    b_L2 = np.asarray(b_L2, dtype=np.float32)

    htt = np.ascontiguousarray(ht.T)                       # [D, B]
    x = np.concatenate([hs, ht], axis=1)                   # [B, 2D]
    xt = np.ascontiguousarray(x.T)                         # [2D, B]
    wlt = np.ascontiguousarray(W_L.T)                      # [2D, 2D]

    in_maps = []
    for c in range(NCORES):
        w2t_c = np.ascontiguousarray(W_L2[:, c * KC : (c + 1) * KC].T)  # [KC, 2D]
        hs_c = hs[:, c * IC : (c + 1) * IC].T               # [IC, B]
        hsb_c = np.ascontiguousarray(
            np.broadcast_to(hs_c[:, None, :], (IC, 128, B))
        )                                                   # [IC, 128, B]
        xt_c = np.ascontiguousarray(xt[:, c * 128 : (c + 1) * 128])  # [2D, 128]
        in_maps.append(
            {"w2t": w2t_c, "hsb": hsb_c, "htt": htt, "xt": xt_c, "wlt": wlt}
        )

    if _NC_CACHE is None:
        _NC_CACHE = _gen()
    nc = _NC_CACHE

    res = run_bass_kernel_spmd(nc, in_maps, core_ids=list(range(NCORES)))
    LAST_EXEC_TIME_NS = res.exec_time_ns
    LAST_RESULTS = res

    ut = np.zeros((TWO_D, B), dtype=np.float64)
    for c in range(NCORES):
        ut += res.results[c]["u_out"]
    u = (ut.T + b_L2).astype(np.float32)

    g = np.concatenate([res.results[c]["g_out"] for c in range(NCORES)], axis=0)
    g = g + b_L

    return np.concatenate([g, u], axis=1).astype(np.float32)


# revision 16
# speedup vs baseline: 1.0224x; 1.0224x over previous
"""DeepAndWide Trainium2 kernel (8 NeuronCores, SPMD via Bass/Tile).

Math:
    g = concat(hs, ht, 1) @ W_L.T + b_L            # [B, 2D] deep path
    v = outer(hs_b, ht_b) flattened -> [B, D*D]    # wide-path features
    u = v @ W_L2.T + b_L2                          # [B, 2D]
    out = concat(g, u, 1)                          # [B, 4D]

Sharding: W_L2 is column-sharded over the D*D contraction dim across 8
cores (each core owns 48 of the 384 outer-product rows i); every core
computes a partial u over the full batch; the host sums the partials.
g is data-parallel over batch (128 rows per core).

On-chip per core: vT tiles ([k,b] layout) are built on the VectorEngine as
htT[j-block] * broadcast(hs[:, i]); the broadcast across partitions is
pre-materialized on the host (HSB). Matmuls run in float32r (full PE rate,
~2e-4 relative error) with W2 chunks stationary ([128k,128o]) and vT
moving (N=512). Output is accumulated transposed (uT) in PSUM, flushed
per 3-i group into an SBUF fp32 accumulator.
"""

import numpy as np

import concourse.bass as bass
import concourse.mybir as mybir
from concourse import tile
from concourse.bass_utils import run_bass_kernel_spmd

B = 1024
D = 384
NCORES = 8
IC = D // NCORES          # 48 outer-product rows (i) per core
KC = IC * D               # 18432 contraction columns per core
TWO_D = 2 * D             # 768

KG_I = 3                  # i's per flush group
NKG = IC // KG_I          # 16 flush groups
TKG = KG_I * 3            # 9 k-tiles (of 128) per flush group

CDT = mybir.dt.float32r   # matmul compute dtype
F32 = mybir.dt.float32

# exec_time_ns of the last hardware run (set when BASS_TRACE=1)
LAST_EXEC_TIME_NS = None
LAST_RESULTS = None


def _split_excess_waits(nc):
    """walrus rejects >1 sync-wait on several instruction structs (fp32/f32r
    Matmult, Drain, ...). Hoist all but the last wait of any multi-wait
    instruction onto single-wait EventSemaphore instructions inserted just
    before it on the same engine."""
    n = [0]

    def fresh():
        n[0] += 1
        return f"WSPLIT-{n[0]}"

    for f in nc.m.functions:
        for blk in f.blocks:
            out = []
            changed = False
            for ins in blk.instructions:
                si = ins.sync_info
                if si is not None and len(si.on_wait) > 1:
                    waits = list(si.on_wait)
                    for w in waits[:-1]:
                        ev = mybir.InstEventSemaphore(
                            name=fresh(),
                            engine=ins.engine,
                            ins=[],
                            outs=[],
                            sync_info=mybir.SyncInfo(on_wait=[w], on_update=[]),
                        )
                        out.append(ev)
                    ins.sync_info = mybir.SyncInfo(
                        on_wait=[waits[-1]], on_update=list(si.on_update)
                    )
                    changed = True
                out.append(ins)
            if changed:
                blk.instructions = out


def _strip_unused_mm_incs(nc):
    """Every matmul carries a +1 update on the PE semaphore; the EVT_SEM
    register write costs the PE queue ~26ns each. Keep only the increments
    whose cumulative tick some wait actually references (plus the final
    one), and renumber all waits on that semaphore accordingly."""
    from collections import defaultdict

    for f in nc.m.functions:
        upd_insts = defaultdict(list)
        wait_refs = defaultdict(list)
        for blk in f.blocks:
            for ins in blk.instructions:
                si = ins.sync_info
                if not si:
                    continue
                for u in si.on_update:
                    upd_insts[u.id].append((ins, u))
                for w in si.on_wait:
                    wait_refs[w.id].append(w)

        for sem_id, upds in upd_insts.items():
            if not all(
                type(i).__name__ == "InstMatmult"
                and u.update_mode == "sem-inc"
                and u.update_value == 1
                for i, u in upds
            ):
                continue
            ws = wait_refs.get(sem_id, [])
            if any(
                w.wait_mode != "sem-ge-imm" or w.wait_reg is not None for w in ws
            ):
                continue
            used = {w.wait_value for w in ws}
            n = len(upds)
            keep = []
            kept_prefix = []
            kept = 0
            for tick in range(1, n + 1):
                k = tick in used or tick == n
                keep.append(k)
                kept += 1 if k else 0
                kept_prefix.append(kept)
            for (ins, u), k in zip(upds, keep):
                if not k:
                    si = ins.sync_info
                    ins.sync_info = mybir.SyncInfo(
                        on_wait=list(si.on_wait),
                        on_update=[x for x in si.on_update if x.id != sem_id],
                    )
            for w in ws:
                v = w.wait_value
                if v >= 1:
                    w.wait_value = kept_prefix[min(v, n) - 1]


def _gen():
    nc = bass.Bass()

    w2t = nc.dram_tensor("w2t", [KC, TWO_D], CDT, kind="ExternalInput")
    hsb = nc.dram_tensor("hsb", [IC, 128, B], CDT, kind="ExternalInput")
    htt = nc.dram_tensor("htt", [D, B], CDT, kind="ExternalInput")
    xt = nc.dram_tensor("xt", [TWO_D, 128], CDT, kind="ExternalInput")
    wlt = nc.dram_tensor("wlt", [TWO_D, TWO_D], CDT, kind="ExternalInput")
    u_out = nc.dram_tensor("u_out", [TWO_D, B], F32, kind="ExternalOutput")  # uT
    g_out = nc.dram_tensor("g_out", [128, TWO_D], F32, kind="ExternalOutput")

    w2t_r = w2t.rearrange("(g t p) o -> g t p o", t=TKG, p=128)   # [NKG,TKG,128,2D]
    hsb_r = hsb.rearrange("(g i) p b -> g i p b", i=KG_I)         # [NKG,KG_I,128,B]
    htt_r = htt.rearrange("(j p) b -> j p b", p=128)              # [3,128,B]
    u_out_r = u_out.rearrange("(c p) b -> c p b", p=128)          # [6,128,B]

    with tile.TileContext(nc) as tc:
        with (
            tc.tile_pool(name="const", bufs=1) as constp,
            tc.tile_pool(name="w2p", bufs=2) as w2p,
            tc.tile_pool(name="hsbp", bufs=2) as hsbp,
            tc.tile_pool(name="vtp", bufs=2) as vtp,
            tc.tile_pool(name="psump", bufs=3, space="PSUM") as psump,
            tc.tile_pool(name="gpool", bufs=1) as gp,
            tc.tile_pool(name="gpsum", bufs=1, space="PSUM") as gps,
        ):
            wlt_t = gp.tile([128, 6, TWO_D], CDT, tag="wlt")
            xt_t = gp.tile([128, 6, 128], CDT, tag="xtt")
            g_sb = gp.tile([128, TWO_D], F32, tag="gsb")

            htt_t = constp.tile([128, 3, B], CDT, tag="htt")
            # uT accumulator: uacc[p, oc, b] = u[b, oc*128 + p]
            uacc = constp.tile([128, 6, B], F32, tag="uacc")

            # critical-path loads first; issue spread over idle engine
            # queues (DMA issue serializes at ~0.6us per op per engine)
            for jb in range(3):
                nc.scalar.dma_start(htt_t[:, jb, :], htt_r[jb])

            # i-group schedule: ramp up so early compute tracks DMA arrival
            groups = [1, 1, 1] + [KG_I] * (NKG - 1)
            assert sum(groups) == IC
            i0 = 0
            for gi, gsz in enumerate(groups):
                nt = gsz * 3                    # k-tiles in this group
                w2_t = w2p.tile([128, nt, TWO_D], CDT, tag="w2")
                hsb_t = hsbp.tile([128, gsz, B], CDT, tag="hsb")
                for il in range(gsz):
                    nc.gpsimd.dma_start(
                        hsb_t[:, il, :],
                        hsb.rearrange("i p b -> i p b")[i0 + il],
                    )
                w2_flat = w2t.rearrange("(kt p) o -> kt p o", p=128)
                if gi < 4:
                    # fine granularity so the first matmuls start early
                    for t in range(nt):
                        nc.sync.dma_start(w2_t[:, t, :], w2_flat[i0 * 3 + t])
                else:
                    nc.sync.dma_start(
                        w2_t[:],
                        w2t.rearrange("(kt p) o -> kt p o", p=128)[
                            i0 * 3 : i0 * 3 + nt
                        ].rearrange("t p o -> p t o"),
                    )
                if gi == 8:
                    # g-path inputs; g compute happens near the end
                    nc.scalar.dma_start(
                        wlt_t[:], wlt.rearrange("(t p) o -> p t o", p=128)
                    )
                    nc.scalar.dma_start(
                        xt_t[:], xt.rearrange("(t p) b -> p t b", p=128)
                    )

                for bh in range(2):
                    bsl = slice(bh * 512, (bh + 1) * 512)
                    vt = vtp.tile([128, nt, 512], CDT, tag="vt")
                    for t in range(nt):
                        il, jb = divmod(t, 3)
                        nc.vector.tensor_mul(
                            vt[:, t, :], htt_t[:, jb, bsl], hsb_t[:, il, bsl]
                        )
                    # lhsT = W2 chunk [k,128o] (stationary), rhs = vT [k,512b]
                    # out = uT chunk [128o, 512b]; PSUM tile holds 2 o-chunks.
                    # k-tiles consumed in chunks of 3 so weight DMA arrival
                    # is just-in-time instead of whole-group bursts.
                    ps0 = psump.tile([128, 2, 512], F32, tag="ps")
                    ps1 = psump.tile([128, 2, 512], F32, tag="ps")
                    ps2 = psump.tile([128, 2, 512], F32, tag="ps")
                    pss = [ps0, ps1, ps2]
                    for tp in range(gsz):
                        for op_ in range(3):
                            for half in range(2):
                                oc = op_ * 2 + half
                                for tt in range(3):
                                    t = tp * 3 + tt
                                    nc.tensor.matmul(
                                        pss[op_][:, half, :],
                                        w2_t[:, t, oc * 128 : (oc + 1) * 128],
                                        vt[:, t, :],
                                        start=(t == 0),
                                        stop=(t == nt - 1),
                                    )
                    for op_ in range(3):
                        if gi == 0:
                            nc.vector.tensor_copy(
                                uacc[:, op_ * 2 : op_ * 2 + 2, bsl], pss[op_][:]
                            )
                        else:
                            nc.vector.tensor_add(
                                uacc[:, op_ * 2 : op_ * 2 + 2, bsl],
                                uacc[:, op_ * 2 : op_ * 2 + 2, bsl],
                                pss[op_][:],
                            )

                if gi == len(groups) - 2:
                    # deep path compute slotted before the last group so it
                    # stays off the kernel tail
                    g_ps = gps.tile([128, TWO_D], F32, tag="gps")
                    for t in range(6):
                        nc.tensor.matmul(
                            g_ps[:, 0:512],
                            xt_t[:, t, :],
                            wlt_t[:, t, 0:512],
                            start=(t == 0),
                            stop=(t == 5),
                        )
                        nc.tensor.matmul(
                            g_ps[:, 512:768],
                            xt_t[:, t, :],
                            wlt_t[:, t, 512:768],
                            start=(t == 0),
                            stop=(t == 5),
                        )
                    nc.vector.tensor_copy(g_sb[:], g_ps[:])
                    nc.sync.dma_start(g_out[:], g_sb[:])
                i0 += gsz

            for oc in range(6):
                nc.sync.dma_start(u_out_r[oc], uacc[:, oc, :])

    _split_excess_waits(nc)
    _strip_unused_mm_incs(nc)
    return nc


_NC_CACHE = None


def kernel(hspatial, htext, W_L, b_L, W_L2, b_L2):
    global LAST_EXEC_TIME_NS, LAST_RESULTS, _NC_CACHE

    hs = np.asarray(hspatial, dtype=np.float32)
    ht = np.asarray(htext, dtype=np.float32)
    W_L = np.asarray(W_L, dtype=np.float32)
    b_L = np.asarray(b_L, dtype=np.float32)
    W_L2 = np.asarray(W_L2, dtype=np.float32)
    b_L2 = np.asarray(b_L2, dtype=np.float32)

    htt = np.ascontiguousarray(ht.T)                       # [D, B]
    x = np.concatenate([hs, ht], axis=1)                   # [B, 2D]
    xt = np.ascontiguousarray(x.T)                         # [2D, B]
    wlt = np.ascontiguousarray(W_L.T)                      # [2D, 2D]

    in_maps = []
    for c in range(NCORES):
        w2t_c = np.ascontiguousarray(W_L2[:, c * KC : (c + 1) * KC].T)  # [KC, 2D]
        hs_c = hs[:, c * IC : (c + 1) * IC].T               # [IC, B]
        hsb_c = np.ascontiguousarray(
            np.broadcast_to(hs_c[:, None, :], (IC, 128, B))
        )                                                   # [IC, 128, B]
        xt_c = np.ascontiguousarray(xt[:, c * 128 : (c + 1) * 128])  # [2D, 128]
        in_maps.append(
            {"w2t": w2t_c, "hsb": hsb_c, "htt": htt, "xt": xt_c, "wlt": wlt}
        )

    if _NC_CACHE is None:
        _NC_CACHE = _gen()
    nc = _NC_CACHE

    res = run_bass_kernel_spmd(nc, in_maps, core_ids=list(range(NCORES)))
    LAST_EXEC_TIME_NS = res.exec_time_ns
    LAST_RESULTS = res

    ut = np.zeros((TWO_D, B), dtype=np.float64)
    for c in range(NCORES):
        ut += res.results[c]["u_out"]
    u = (ut.T + b_L2).astype(np.float32)

    g = np.concatenate([res.results[c]["g_out"] for c in range(NCORES)], axis=0)
    g = g + b_L

    return np.concatenate([g, u], axis=1).astype(np.float32)


# revision 17
# speedup vs baseline: 1.0974x; 1.0733x over previous
"""DeepAndWide Trainium2 kernel (8 NeuronCores, SPMD via Bass/Tile).

Math:
    g = concat(hs, ht, 1) @ W_L.T + b_L            # [B, 2D] deep path
    v = outer(hs_b, ht_b) flattened -> [B, D*D]    # wide-path features
    u = v @ W_L2.T + b_L2                          # [B, 2D]
    out = concat(g, u, 1)                          # [B, 4D]

Sharding: W_L2 is column-sharded over the D*D contraction dim across 8
cores (each core owns 48 of the 384 outer-product rows i); every core
computes a partial u over the full batch; the host sums the partials.
g is data-parallel over batch (128 rows per core).

On-chip per core: vT tiles ([k,b] layout) are built on the VectorEngine as
htT[j-block] * broadcast(hs[:, i]); the broadcast across partitions is
pre-materialized on the host (HSB). Matmuls run in float32r (full PE rate,
~2e-4 relative error) with W2 chunks stationary ([128k,128o]) and vT
moving (N=512). Output is accumulated transposed (uT) in PSUM, flushed
per 3-i group into an SBUF fp32 accumulator.
"""

import numpy as np

import concourse.bass as bass
import concourse.mybir as mybir
from concourse import tile
from concourse.bass_utils import run_bass_kernel_spmd

B = 1024
D = 384
NCORES = 8
IC = D // NCORES          # 48 outer-product rows (i) per core
KC = IC * D               # 18432 contraction columns per core
TWO_D = 2 * D             # 768

KG_I = 3                  # i's per flush group
NKG = IC // KG_I          # 16 flush groups
TKG = KG_I * 3            # 9 k-tiles (of 128) per flush group

import os as _os
_DTYPE_NAME = _os.environ.get("KERNEL_DTYPE", "f32r")
CDT = {"f32r": mybir.dt.float32r, "fp16": mybir.dt.float16, "bf16": mybir.dt.bfloat16}[
    _DTYPE_NAME
]
_NP_CDT = {"f32r": np.float32, "fp16": np.float16, "bf16": None}[_DTYPE_NAME]
F32 = mybir.dt.float32

# exec_time_ns of the last hardware run (set when BASS_TRACE=1)
LAST_EXEC_TIME_NS = None
LAST_RESULTS = None


def _split_excess_waits(nc):
    """walrus rejects >1 sync-wait on several instruction structs (fp32/f32r
    Matmult, Drain, ...). Hoist all but the last wait of any multi-wait
    instruction onto single-wait EventSemaphore instructions inserted just
    before it on the same engine."""
    n = [0]

    def fresh():
        n[0] += 1
        return f"WSPLIT-{n[0]}"

    for f in nc.m.functions:
        for blk in f.blocks:
            out = []
            changed = False
            for ins in blk.instructions:
                si = ins.sync_info
                if si is not None and len(si.on_wait) > 1:
                    waits = list(si.on_wait)
                    for w in waits[:-1]:
                        ev = mybir.InstEventSemaphore(
                            name=fresh(),
                            engine=ins.engine,
                            ins=[],
                            outs=[],
                            sync_info=mybir.SyncInfo(on_wait=[w], on_update=[]),
                        )
                        out.append(ev)
                    ins.sync_info = mybir.SyncInfo(
                        on_wait=[waits[-1]], on_update=list(si.on_update)
                    )
                    changed = True
                out.append(ins)
            if changed:
                blk.instructions = out


def _strip_unused_mm_incs(nc):
    """Every matmul carries a +1 update on the PE semaphore; the EVT_SEM
    register write costs the PE queue ~26ns each. Keep only the increments
    whose cumulative tick some wait actually references (plus the final
    one), and renumber all waits on that semaphore accordingly."""
    from collections import defaultdict

    for f in nc.m.functions:
        upd_insts = defaultdict(list)
        wait_refs = defaultdict(list)
        for blk in f.blocks:
            for ins in blk.instructions:
                si = ins.sync_info
                if not si:
                    continue
                for u in si.on_update:
                    upd_insts[u.id].append((ins, u))
                for w in si.on_wait:
                    wait_refs[w.id].append(w)

        for sem_id, upds in upd_insts.items():
            if not all(
                type(i).__name__ == "InstMatmult"
                and u.update_mode == "sem-inc"
                and u.update_value == 1
                for i, u in upds
            ):
                continue
            ws = wait_refs.get(sem_id, [])
            if any(
                w.wait_mode != "sem-ge-imm" or w.wait_reg is not None for w in ws
            ):
                continue
            used = {w.wait_value for w in ws}
            n = len(upds)
            keep = []
            kept_prefix = []
            kept = 0
            for tick in range(1, n + 1):
                k = tick in used or tick == n
                keep.append(k)
                kept += 1 if k else 0
                kept_prefix.append(kept)
            for (ins, u), k in zip(upds, keep):
                if not k:
                    si = ins.sync_info
                    ins.sync_info = mybir.SyncInfo(
                        on_wait=list(si.on_wait),
                        on_update=[x for x in si.on_update if x.id != sem_id],
                    )
            for w in ws:
                v = w.wait_value
                if v >= 1:
                    w.wait_value = kept_prefix[min(v, n) - 1]


def _gen():
    nc = bass.Bass()

    w2t = nc.dram_tensor("w2t", [KC, TWO_D], CDT, kind="ExternalInput")
    hsb = nc.dram_tensor("hsb", [IC, 128, B], CDT, kind="ExternalInput")
    htt = nc.dram_tensor("htt", [D, B], CDT, kind="ExternalInput")
    xt = nc.dram_tensor("xt", [TWO_D, 128], CDT, kind="ExternalInput")
    wlt = nc.dram_tensor("wlt", [TWO_D, TWO_D], CDT, kind="ExternalInput")
    u_out = nc.dram_tensor("u_out", [TWO_D, B], F32, kind="ExternalOutput")  # uT
    g_out = nc.dram_tensor("g_out", [128, TWO_D], F32, kind="ExternalOutput")

    w2t_r = w2t.rearrange("(g t p) o -> g t p o", t=TKG, p=128)   # [NKG,TKG,128,2D]
    hsb_r = hsb.rearrange("(g i) p b -> g i p b", i=KG_I)         # [NKG,KG_I,128,B]
    htt_r = htt.rearrange("(j p) b -> j p b", p=128)              # [3,128,B]
    u_out_r = u_out.rearrange("(c p) b -> c p b", p=128)          # [6,128,B]

    with tile.TileContext(nc) as tc:
        with (
            tc.tile_pool(name="const", bufs=1) as constp,
            tc.tile_pool(name="w2p", bufs=2) as w2p,
            tc.tile_pool(name="hsbp", bufs=2) as hsbp,
            tc.tile_pool(name="vtp", bufs=2) as vtp,
            tc.tile_pool(name="psump", bufs=3, space="PSUM") as psump,
            tc.tile_pool(name="gpool", bufs=1) as gp,
            tc.tile_pool(name="gpsum", bufs=1, space="PSUM") as gps,
        ):
            wlt_t = gp.tile([128, 6, TWO_D], CDT, tag="wlt")
            xt_t = gp.tile([128, 6, 128], CDT, tag="xtt")
            g_sb = gp.tile([128, TWO_D], F32, tag="gsb")

            htt_t = constp.tile([128, 3, B], CDT, tag="htt")
            # uT accumulator: uacc[p, oc, b] = u[b, oc*128 + p]
            uacc = constp.tile([128, 6, B], F32, tag="uacc")

            # critical-path loads first; issue spread over idle engine
            # queues (DMA issue serializes at ~0.6us per op per engine)
            for jb in range(3):
                nc.scalar.dma_start(htt_t[:, jb, :], htt_r[jb])

            # i-group schedule: ramp up so early compute tracks DMA arrival
            groups = [1, 1, 1] + [KG_I] * (NKG - 1)
            assert sum(groups) == IC
            i0 = 0
            for gi, gsz in enumerate(groups):
                nt = gsz * 3                    # k-tiles in this group
                w2_t = w2p.tile([128, nt, TWO_D], CDT, tag="w2")
                hsb_t = hsbp.tile([128, gsz, B], CDT, tag="hsb")
                for il in range(gsz):
                    nc.gpsimd.dma_start(
                        hsb_t[:, il, :],
                        hsb.rearrange("i p b -> i p b")[i0 + il],
                    )
                w2_flat = w2t.rearrange("(kt p) o -> kt p o", p=128)
                if gi < 4:
                    # fine granularity so the first matmuls start early
                    for t in range(nt):
                        nc.sync.dma_start(w2_t[:, t, :], w2_flat[i0 * 3 + t])
                else:
                    nc.sync.dma_start(
                        w2_t[:],
                        w2t.rearrange("(kt p) o -> kt p o", p=128)[
                            i0 * 3 : i0 * 3 + nt
                        ].rearrange("t p o -> p t o"),
                    )
                if gi == 8:
                    # g-path inputs; g compute happens near the end
                    nc.scalar.dma_start(
                        wlt_t[:], wlt.rearrange("(t p) o -> p t o", p=128)
                    )
                    nc.scalar.dma_start(
                        xt_t[:], xt.rearrange("(t p) b -> p t b", p=128)
                    )

                for bh in range(2):
                    bsl = slice(bh * 512, (bh + 1) * 512)
                    vt = vtp.tile([128, nt, 512], CDT, tag="vt")
                    for t in range(nt):
                        il, jb = divmod(t, 3)
                        nc.vector.tensor_mul(
                            vt[:, t, :], htt_t[:, jb, bsl], hsb_t[:, il, bsl]
                        )
                    # lhsT = W2 chunk [k,128o] (stationary), rhs = vT [k,512b]
                    # out = uT chunk [128o, 512b]; PSUM tile holds 2 o-chunks.
                    # k-tiles consumed in chunks of 3 so weight DMA arrival
                    # is just-in-time instead of whole-group bursts.
                    ps0 = psump.tile([128, 2, 512], F32, tag="ps")
                    ps1 = psump.tile([128, 2, 512], F32, tag="ps")
                    ps2 = psump.tile([128, 2, 512], F32, tag="ps")
                    pss = [ps0, ps1, ps2]
                    for tp in range(gsz):
                        for op_ in range(3):
                            for half in range(2):
                                oc = op_ * 2 + half
                                for tt in range(3):
                                    t = tp * 3 + tt
                                    nc.tensor.matmul(
                                        pss[op_][:, half, :],
                                        w2_t[:, t, oc * 128 : (oc + 1) * 128],
                                        vt[:, t, :],
                                        start=(t == 0),
                                        stop=(t == nt - 1),
                                    )
                    for op_ in range(3):
                        if gi == 0:
                            nc.vector.tensor_copy(
                                uacc[:, op_ * 2 : op_ * 2 + 2, bsl], pss[op_][:]
                            )
                        else:
                            nc.vector.tensor_add(
                                uacc[:, op_ * 2 : op_ * 2 + 2, bsl],
                                uacc[:, op_ * 2 : op_ * 2 + 2, bsl],
                                pss[op_][:],
                            )

                if gi == len(groups) - 2:
                    # deep path compute slotted before the last group so it
                    # stays off the kernel tail
                    g_ps = gps.tile([128, TWO_D], F32, tag="gps")
                    for t in range(6):
                        nc.tensor.matmul(
                            g_ps[:, 0:512],
                            xt_t[:, t, :],
                            wlt_t[:, t, 0:512],
                            start=(t == 0),
                            stop=(t == 5),
                        )
                        nc.tensor.matmul(
                            g_ps[:, 512:768],
                            xt_t[:, t, :],
                            wlt_t[:, t, 512:768],
                            start=(t == 0),
                            stop=(t == 5),
                        )
                    nc.vector.tensor_copy(g_sb[:], g_ps[:])
                    nc.sync.dma_start(g_out[:], g_sb[:])
                i0 += gsz

            for oc in range(6):
                nc.sync.dma_start(u_out_r[oc], uacc[:, oc, :])

    _split_excess_waits(nc)
    _strip_unused_mm_incs(nc)
    return nc


_NC_CACHE = None


def kernel(hspatial, htext, W_L, b_L, W_L2, b_L2):
    global LAST_EXEC_TIME_NS, LAST_RESULTS, _NC_CACHE

    hs = np.asarray(hspatial, dtype=np.float32)
    ht = np.asarray(htext, dtype=np.float32)
    W_L = np.asarray(W_L, dtype=np.float32)
    b_L = np.asarray(b_L, dtype=np.float32)
    W_L2 = np.asarray(W_L2, dtype=np.float32)
    b_L2 = np.asarray(b_L2, dtype=np.float32)

    cdt = _NP_CDT
    htt = np.ascontiguousarray(ht.T.astype(cdt))           # [D, B]
    x = np.concatenate([hs, ht], axis=1)                   # [B, 2D]
    xt = np.ascontiguousarray(x.T.astype(cdt))             # [2D, B]
    wlt = np.ascontiguousarray(W_L.T.astype(cdt))          # [2D, 2D]

    in_maps = []
    for c in range(NCORES):
        w2t_c = np.ascontiguousarray(
            W_L2[:, c * KC : (c + 1) * KC].T.astype(cdt)
        )                                                   # [KC, 2D]
        hs_c = hs[:, c * IC : (c + 1) * IC].T.astype(cdt)   # [IC, B]
        hsb_c = np.ascontiguousarray(
            np.broadcast_to(hs_c[:, None, :], (IC, 128, B))
        )                                                   # [IC, 128, B]
        xt_c = np.ascontiguousarray(xt[:, c * 128 : (c + 1) * 128])  # [2D, 128]
        in_maps.append(
            {"w2t": w2t_c, "hsb": hsb_c, "htt": htt, "xt": xt_c, "wlt": wlt}
        )

    if _NC_CACHE is None:
        _NC_CACHE = _gen()
    nc = _NC_CACHE

    res = run_bass_kernel_spmd(nc, in_maps, core_ids=list(range(NCORES)))
    LAST_EXEC_TIME_NS = res.exec_time_ns
    LAST_RESULTS = res

    ut = np.zeros((TWO_D, B), dtype=np.float64)
    for c in range(NCORES):
        ut += res.results[c]["u_out"]
    u = (ut.T + b_L2).astype(np.float32)

    g = np.concatenate([res.results[c]["g_out"] for c in range(NCORES)], axis=0)
    g = g + b_L

    return np.concatenate([g, u], axis=1).astype(np.float32)
